# revision 1
# baseline (speedup 1.0000x reference)
"""Trainium2 Bass kernel for ConvReverseDataNet (USRNet-style FFT data step).

Math (per (b,c) plane, sf=2, validated vs reference in fp32):
  g   = fft2_128(x)                                (128x128 complex)
  FB  = G @ k @ G^T, G = F256[:, roll_idx]         (256x256 complex)
  W   = sum_{4 blocks} |FB|^2 ;  Y0 = sum_{4 blocks} FB*DD   (128x128)
  wt  = (4 - Y0) / (W + 4*be)                      (128x128 complex)
  FX  = tile(g) * (conj(FB)*tile(wt) + DD)         (256x256 complex)
  out = real(ifft2_256(FX))                         = Fc@FX@Fc / 65536
where DD = outer(d, d), d[t] = 1 + exp(-2*pi*i*t/256), be = sigmoid(alpha-9)+1e-3.
All complex arrays X are stored as (Xr, Xs) with X = Xr - i*Xs.
256x256 planes live in SBUF as [128, 512]: [p, hb*256+f] = plane[hb*128+p, f].

Sharding: 256 (b,c) planes over 8 cores; core ci gets channels ci*8..ci*8+7 x all 4 batches.
"""

import functools
import sys

import numpy as np

if "/opt/trn_rl_repo" not in sys.path:
    sys.path.insert(0, "/opt/trn_rl_repo")

from concourse import bacc, bass, mybir, tile  # noqa: E402
from concourse.bass_utils import run_bass_kernel_spmd  # noqa: E402

F32 = mybir.dt.float32
MULT = mybir.AluOpType.mult
ADD = mybir.AluOpType.add

N_CORES = 8
NPL = 32  # planes per core
KS = 25


def _host_consts():
    t1 = np.arange(128)
    th1 = 2 * np.pi * np.outer(t1, t1) / 128
    C1 = np.cos(th1).astype(np.float32)
    S1 = np.sin(th1).astype(np.float32)
    t2 = np.arange(256)
    th2 = 2 * np.pi * np.outer(t2, t2) / 256
    C2 = np.cos(th2)
    S2 = np.sin(th2)
    idx = (np.arange(KS) - (KS // 2)) % 256
    GcT = C2[idx, :].astype(np.float32)  # [25,256]
    GsT = S2[idx, :].astype(np.float32)
    # Cnat[p, kc*256+m] = C2[kc*128+p, m]
    Cnat = C2.reshape(2, 128, 256).transpose(1, 0, 2).reshape(128, 512).astype(np.float32)
    Snat = S2.reshape(2, 128, 256).transpose(1, 0, 2).reshape(128, 512).astype(np.float32)
    dr = 1 + np.cos(2 * np.pi * t2 / 256)
    ds = np.sin(2 * np.pi * t2 / 256)

    def to_plane(a):
        return a.reshape(2, 128, 256).transpose(1, 0, 2).reshape(128, 512)

    DDr = to_plane(np.outer(dr, dr) - np.outer(ds, ds)).astype(np.float32)
    DDs = to_plane(np.outer(dr, ds) + np.outer(ds, dr)).astype(np.float32)
    return {
        "C1": C1, "S1": S1, "S1n": -S1,
        "GcT": GcT, "GsT": GsT, "GsTn": -GsT,
        "Cnat": Cnat, "Snat": Snat, "Snatn": -Snat,
        "DDr": DDr, "DDs": DDs,
    }


CONST_SHAPES = {
    "C1": [128, 128], "S1": [128, 128], "S1n": [128, 128],
    "GcT": [KS, 256], "GsT": [KS, 256], "GsTn": [KS, 256],
    "Cnat": [128, 512], "Snat": [128, 512], "Snatn": [128, 512],
    "DDr": [128, 512], "DDs": [128, 512],
}


def build_nc(n_planes=NPL):
    nc = bacc.Bacc("TRN2", target_bir_lowering=False, debug=False, num_devices=N_CORES)

    xs_t = nc.dram_tensor("xs", [n_planes, 128, 128], F32, kind="ExternalInput")
    kt_t = nc.dram_tensor("kt", [n_planes, KS, KS], F32, kind="ExternalInput")
    be4_t = nc.dram_tensor("be4", [128, n_planes], F32, kind="ExternalInput")
    const_t = {n: nc.dram_tensor(n, s, F32, kind="ExternalInput") for n, s in CONST_SHAPES.items()}
    out_t = nc.dram_tensor("out", [n_planes, 256, 256], F32, kind="ExternalOutput")

    with tile.TileContext(nc) as tc:
        with (
            tc.tile_pool(name="cpool", bufs=1) as cpool,
            tc.tile_pool(name="small", bufs=3) as small,
            tc.tile_pool(name="big", bufs=2) as big,
            tc.tile_pool(name="psum", bufs=1, space="PSUM") as pp,
        ):
            cs = {}
            for n, s in CONST_SHAPES.items():
                cs[n] = cpool.tile(s, F32, tag=n, name=f"c_{n}")
                nc.sync.dma_start(cs[n][:], const_t[n][:])
            be4sb = cpool.tile([128, n_planes], F32, tag="be4sb")
            nc.sync.dma_start(be4sb[:], be4_t[:])

            def b4(ap):  # view flat [128,512] as [128,4,128]
                return ap.rearrange("p (b f) -> p b f", b=4)

            def rep4(ap128):  # [128,128] -> broadcast [128,4,128]
                return ap128.unsqueeze(1).broadcast_to([128, 4, 128])

            for i in range(n_planes):
                # ---- loads ----
                x_sb = small.tile([128, 128], F32, tag="x_sb")
                nc.sync.dma_start(x_sb[:], xs_t[i])
                kt_sb = small.tile([KS, KS], F32, tag="kt_sb")
                nc.sync.dma_start(kt_sb[:], kt_t[i])

                # ---- fft128: g = F1 @ x @ F1 ----
                z_sb = small.tile([128, 256], F32, tag="z_sb")  # Zrt | Zst
                pzr = pp.tile([128, 128], F32, tag="p128", bufs=2)
                nc.tensor.matmul(pzr[:], x_sb[:], cs["C1"][:], start=True, stop=True)
                nc.scalar.copy(z_sb[:, 0:128], pzr[:])
                pzs = pp.tile([128, 128], F32, tag="p128", bufs=2)
                nc.tensor.matmul(pzs[:], x_sb[:], cs["S1"][:], start=True, stop=True)
                nc.scalar.copy(z_sb[:, 128:256], pzs[:])

                g_sb = small.tile([128, 256], F32, tag="g_sb")  # gr | gs
                pgr = pp.tile([128, 128], F32, tag="p128", bufs=2)
                nc.tensor.matmul(pgr[:], z_sb[:, 0:128], cs["C1"][:], start=True, stop=False)
                nc.tensor.matmul(pgr[:], z_sb[:, 128:256], cs["S1n"][:], start=False, stop=True)
                nc.scalar.copy(g_sb[:, 0:128], pgr[:])
                pgs = pp.tile([128, 128], F32, tag="p128", bufs=2)
                nc.tensor.matmul(pgs[:], z_sb[:, 0:128], cs["S1"][:], start=True, stop=False)
                nc.tensor.matmul(pgs[:], z_sb[:, 128:256], cs["C1"][:], start=False, stop=True)
                nc.scalar.copy(g_sb[:, 128:256], pgs[:])

                # ---- FB = G @ k @ G^T ----
                a_sb = small.tile([KS, 512], F32, tag="a_sb")  # Ar | As
                par = pp.tile([KS, 256], F32, tag="pa")
                nc.tensor.matmul(par[:], kt_sb[:], cs["GcT"][:], start=True, stop=True)
                nc.scalar.copy(a_sb[:, 0:256], par[:])
                pas = pp.tile([KS, 256], F32, tag="pa")
                nc.tensor.matmul(pas[:], kt_sb[:], cs["GsT"][:], start=True, stop=True)
                nc.scalar.copy(a_sb[:, 256:512], pas[:])

                FBr = big.tile([128, 512], F32, tag="FBr")
                FBs = big.tile([128, 512], F32, tag="FBs")
                for hh in range(2):
                    hsl = slice(hh * 128, (hh + 1) * 128)
                    pfbr = pp.tile([128, 256], F32, tag="pfb", bufs=2)
                    nc.tensor.matmul(pfbr[:], cs["GcT"][:, hsl], a_sb[:, 0:256], start=True, stop=False)
                    nc.tensor.matmul(pfbr[:], cs["GsTn"][:, hsl], a_sb[:, 256:512], start=False, stop=True)
                    nc.scalar.copy(FBr[:, hh * 256:(hh + 1) * 256], pfbr[:])
                    pfbs = pp.tile([128, 256], F32, tag="pfb", bufs=2)
                    nc.tensor.matmul(pfbs[:], cs["GcT"][:, hsl], a_sb[:, 256:512], start=True, stop=False)
                    nc.tensor.matmul(pfbs[:], cs["GsT"][:, hsl], a_sb[:, 0:256], start=False, stop=True)
                    nc.scalar.copy(FBs[:, hh * 256:(hh + 1) * 256], pfbs[:])

                # ---- elementwise: W, Y0, wt ----
                sq1 = big.tile([128, 512], F32, tag="sq1")
                sq2 = big.tile([128, 512], F32, tag="sq2")
                nc.scalar.square(sq1[:], FBr[:])
                nc.scalar.square(sq2[:], FBs[:])
                F2B = big.tile([128, 512], F32, tag="F2B")
                nc.vector.tensor_add(F2B[:], sq1[:], sq2[:])

                m1 = big.tile([128, 512], F32, tag="m1")
                m2 = big.tile([128, 512], F32, tag="m2")
                nc.vector.tensor_mul(m1[:], FBr[:], cs["DDr"][:])
                nc.vector.scalar_tensor_tensor(m2[:], FBs[:], -1.0, cs["DDs"][:], MULT, MULT)
                Pr0 = big.tile([128, 512], F32, tag="Pr0")
                nc.vector.tensor_add(Pr0[:], m1[:], m2[:])
                m3 = big.tile([128, 512], F32, tag="m3")
                m4 = big.tile([128, 512], F32, tag="m4")
                nc.vector.tensor_mul(m3[:], FBr[:], cs["DDs"][:])
                nc.vector.tensor_mul(m4[:], FBs[:], cs["DDr"][:])
                Ps0 = big.tile([128, 512], F32, tag="Ps0")
                nc.vector.tensor_add(Ps0[:], m3[:], m4[:])

                sums = {}
                for nm, src in (("W", F2B), ("Yr0", Pr0), ("Ys0", Ps0)):
                    sA = small.tile([128, 256], F32, tag=f"sA_{nm}")
                    v = src[:].rearrange("p (a b f) -> p a b f", a=2, b=2)
                    nc.vector.tensor_add(sA[:].rearrange("p (a f) -> p a f", a=2), v[:, :, 0, :], v[:, :, 1, :])
                    dst = small.tile([128, 128], F32, tag=nm)
                    nc.vector.tensor_add(dst[:], sA[:, 0:128], sA[:, 128:256])
                    sums[nm] = dst

                den = small.tile([128, 128], F32, tag="den")
                nc.vector.tensor_scalar_add(den[:], sums["W"][:], be4sb[:, i:i + 1])
                dinv = small.tile([128, 128], F32, tag="dinv")
                nc.vector.reciprocal_approx_fast(dinv[:], den[:])
                wt4 = small.tile([128, 128], F32, tag="wt4")
                nc.vector.tensor_scalar(wt4[:], sums["Yr0"][:], -1.0, 4.0, MULT, ADD)
                wr = small.tile([128, 128], F32, tag="wr")
                nc.vector.tensor_mul(wr[:], wt4[:], dinv[:])
                ws = small.tile([128, 128], F32, tag="ws")
                nc.vector.scalar_tensor_tensor(ws[:], sums["Ys0"][:], -1.0, dinv[:], MULT, MULT)

                # ---- H = conj(FB) * tile(wt) + DD ----
                twr = rep4(wr[:])
                tws = rep4(ws[:])
                p1 = big.tile([128, 512], F32, tag="p1")
                p2 = big.tile([128, 512], F32, tag="p2")
                nc.vector.tensor_mul(b4(p1[:]), b4(FBr[:]), twr)
                nc.vector.tensor_mul(b4(p2[:]), b4(FBs[:]), tws)
                s12 = big.tile([128, 512], F32, tag="s12")
                nc.vector.tensor_add(s12[:], p1[:], p2[:])
                Hr = big.tile([128, 512], F32, tag="Hr")
                nc.vector.tensor_add(Hr[:], s12[:], cs["DDr"][:])
                p3 = big.tile([128, 512], F32, tag="p3")
                p4 = big.tile([128, 512], F32, tag="p4")
                nc.vector.tensor_mul(b4(p3[:]), b4(FBr[:]), tws)
                nc.vector.scalar_tensor_tensor(b4(p4[:]), b4(FBs[:]), -1.0, twr, MULT, MULT)
                s34 = big.tile([128, 512], F32, tag="s34")
                nc.vector.tensor_add(s34[:], p3[:], p4[:])
                Hs = big.tile([128, 512], F32, tag="Hs")
                nc.vector.tensor_add(Hs[:], s34[:], cs["DDs"][:])

                # ---- FX = tile(g) * H ----
                tgr = rep4(g_sb[:, 0:128])
                tgs = rep4(g_sb[:, 128:256])
                q1 = big.tile([128, 512], F32, tag="q1")
                q2 = big.tile([128, 512], F32, tag="q2")
                nc.vector.tensor_mul(b4(q1[:]), b4(Hr[:]), tgr)
                nc.vector.scalar_tensor_tensor(b4(q2[:]), b4(Hs[:]), -1.0, tgs, MULT, MULT)
                FXr = big.tile([128, 512], F32, tag="FXr")
                nc.vector.tensor_add(FXr[:], q1[:], q2[:])
                q3 = big.tile([128, 512], F32, tag="q3")
                q4 = big.tile([128, 512], F32, tag="q4")
                nc.vector.tensor_mul(b4(q3[:]), b4(Hs[:]), tgr)
                nc.vector.tensor_mul(b4(q4[:]), b4(Hr[:]), tgs)
                FXs = big.tile([128, 512], F32, tag="FXs")
                nc.vector.tensor_add(FXs[:], q3[:], q4[:])

                # ---- ifft stage 1: VT ----
                VTr = big.tile([128, 512], F32, tag="VTr")
                VTs = big.tile([128, 512], F32, tag="VTs")
                for fb in range(2):
                    pvtr = pp.tile([128, 256], F32, tag="pvt", bufs=2)
                    pvts = pp.tile([128, 256], F32, tag="pvt", bufs=2)
                    for kc in range(2):
                        lsl = slice(kc * 256 + fb * 128, kc * 256 + (fb + 1) * 128)
                        csl = slice(kc * 256, (kc + 1) * 256)
                        st = kc == 0
                        nc.tensor.matmul(pvtr[:], FXr[:, lsl], cs["Cnat"][:, csl], start=st, stop=False)
                        nc.tensor.matmul(pvtr[:], FXs[:, lsl], cs["Snat"][:, csl], start=False, stop=(kc == 1))
                    for kc in range(2):
                        lsl = slice(kc * 256 + fb * 128, kc * 256 + (fb + 1) * 128)
                        csl = slice(kc * 256, (kc + 1) * 256)
                        st = kc == 0
                        nc.tensor.matmul(pvts[:], FXs[:, lsl], cs["Cnat"][:, csl], start=st, stop=False)
                        nc.tensor.matmul(pvts[:], FXr[:, lsl], cs["Snatn"][:, csl], start=False, stop=(kc == 1))
                    nc.scalar.copy(VTr[:, fb * 256:(fb + 1) * 256], pvtr[:])
                    nc.scalar.copy(VTs[:, fb * 256:(fb + 1) * 256], pvts[:])

                # ---- ifft stage 2 (real part) + scale + store ----
                out_sb = big.tile([128, 512], F32, tag="out_sb")
                po = pp.tile([128, 512], F32, tag="po")
                for mb in range(2):
                    osl = slice(mb * 256, (mb + 1) * 256)
                    for fb in range(2):
                        lsl = slice(fb * 256 + mb * 128, fb * 256 + (mb + 1) * 128)
                        csl = slice(fb * 256, (fb + 1) * 256)
                        nc.tensor.matmul(po[:, osl], VTr[:, lsl], cs["Cnat"][:, csl], start=(fb == 0), stop=False)
                        nc.tensor.matmul(po[:, osl], VTs[:, lsl], cs["Snat"][:, csl], start=False, stop=(fb == 1))
                    nc.scalar.mul(out_sb[:, osl], po[:, osl], 1.0 / 65536.0)
                nc.sync.dma_start(
                    out_t[i].rearrange("(hb p) f -> p hb f", p=128),
                    out_sb[:].rearrange("p (hb f) -> p hb f", hb=2),
                )

    nc.compile()
    return nc


@functools.lru_cache(maxsize=2)
def _built(n_planes=NPL):
    return build_nc(n_planes)


def make_in_maps(x, k, alpha, n_planes=NPL, n_cores=N_CORES):
    consts = _host_consts()
    alpha_c = alpha.reshape(-1).astype(np.float64)  # [64]
    be = (1.0 / (1.0 + np.exp(-(alpha_c - 9.0))) + 1e-3).astype(np.float32)
    cpc = n_planes // 4  # channels per core
    in_maps = []
    for ci in range(n_cores):
        chs = slice(ci * cpc, (ci + 1) * cpc)
        xs = np.ascontiguousarray(x[:, chs].transpose(1, 0, 2, 3).reshape(n_planes, 128, 128))
        kt = np.ascontiguousarray(k[:, chs].transpose(1, 0, 3, 2).reshape(n_planes, KS, KS))
        be_pl = np.repeat(be[chs], 4)  # plane order: (c_loc, b)
        be4 = np.broadcast_to(4.0 * be_pl, (128, n_planes)).astype(np.float32).copy()
        m = {"xs": xs, "kt": kt, "be4": be4}
        m.update(consts)
        in_maps.append(m)
    return in_maps


def kernel(x, k, alpha, sf=2, **_ignored):
    x = np.asarray(x, dtype=np.float32)
    k = np.asarray(k, dtype=np.float32)
    alpha = np.asarray(alpha, dtype=np.float32)
    assert int(sf) == 2 and x.shape == (4, 64, 128, 128) and k.shape == (4, 64, KS, KS)

    nc = _built(NPL)
    in_maps = make_in_maps(x, k, alpha)
    res = run_bass_kernel_spmd(nc, in_maps, core_ids=list(range(N_CORES)))
    out = np.empty((4, 64, 256, 256), np.float32)
    cpc = NPL // 4
    for ci in range(N_CORES):
        o = res.results[ci]["out"].reshape(cpc, 4, 256, 256).transpose(1, 0, 2, 3)
        out[:, ci * cpc:(ci + 1) * cpc] = o
    return out


if __name__ == "__main__":
    rng = np.random.default_rng(0)
    x = rng.standard_normal((4, 64, 128, 128), dtype=np.float32)
    k = rng.random((4, 64, KS, KS), dtype=np.float32)
    alpha = np.zeros((1, 64, 1, 1), np.float32)
    out = kernel(x, k, alpha, 2)
    print("out", out.shape, out.dtype, float(np.abs(out).max()))



# revision 11
# speedup vs baseline: 1.3310x; 1.3310x over previous
"""Trainium2 Bass kernel for ConvReverseDataNet (USRNet-style FFT data step).

128-grid reformulation (per (b,c) plane, sf=2), validated in fp64/bf16 numpy:
  g   = DFT2_128(x)
  s_pq[a,b] = rolled_psf256[2a+p, 2b+q]   (parity subkernels of the 25x25 psf)
  Khat_pq = DFT2_128(s_pq) * e^{-2pi i(u p + v q)/128}  (twiddles folded into
            the small-DFT constants via positions (j+par)/2)
  W'  = sum_pq |Khat_pq|^2 ;  Y0' = sum_pq Khat_pq      (both on 128 grid)
  wt  = (1 - Y0') / (W' + be) ;  v = g * wt
  out[2m+p, 2n+q] = real(IDFT2_128(conj(Khat_pq) * v))[m,n] + x[m,n]

mm(stat, mov) = stat.T @ mov (contract over partition dim). Complex arrays
X stored as adjacent [Xr | Xs] blocks. Big-DFT matmuls run in float32r
(1 cycle/row at N>=256); the M/ifft stages run in bf16 (validated 3.2e-3).

Sharding: 256 (b,c) planes over 8 cores; core ci gets channels ci*8..ci*8+7.
Output is parity-planar [plane, 2p+q, m, n]; host interleaves.
"""

import functools
import sys

import numpy as np

if "/opt/trn_rl_repo" not in sys.path:
    sys.path.insert(0, "/opt/trn_rl_repo")

from concourse import bacc, bass, mybir, tile  # noqa: E402
from concourse.bass_utils import run_bass_kernel_spmd  # noqa: E402

F32 = mybir.dt.float32
F32R = mybir.dt.float32r
BF16 = mybir.dt.bfloat16
MULT = mybir.AluOpType.mult
ADD = mybir.AluOpType.add
ACT_COPY = mybir.ActivationFunctionType.Copy

N_CORES = 8
NPL = 32  # planes per core
KS = 25


def _host_consts():
    t = np.arange(KS)
    j = (t - 12) % 256            # position in rolled 256 grid
    par = t % 2                   # parity (row p / col q)
    bp = (j + par) // 2           # twiddle-folded 128-grid position

    u = np.arange(128)
    ang = 2 * np.pi * np.outer(bp, u) / 128  # [25,128]
    COSg = np.cos(ang).astype(np.float32)
    SINg = np.sin(ang).astype(np.float32)
    sel = [(par == 0).astype(np.float32)[:, None], (par == 1).astype(np.float32)[:, None]]

    c = {}
    c["WMOV"] = np.concatenate(
        [COSg * sel[0], COSg * sel[1], -SINg * sel[0], -SINg * sel[1]], axis=1
    )  # [25,512] -> A blocks [Ar0|Ar1|As0|As1]
    for p in (0, 1):
        c[f"ZC{p}"] = COSg * sel[p]
        c[f"ZS{p}"] = -SINg * sel[p]
        c[f"ZSn{p}"] = SINg * sel[p]

    th = 2 * np.pi * np.outer(u, u) / 128
    C1 = np.cos(th).astype(np.float32)
    S1 = np.sin(th).astype(np.float32)
    c["CS"] = np.concatenate([C1, S1], 1)
    c["CSn"] = np.concatenate([C1, -S1], 1)
    c["SnCn"] = np.concatenate([-S1, -C1], 1)
    c["ICSf"] = np.concatenate([C1, S1], 1)
    c["ISnCf"] = np.concatenate([-S1, C1], 1)
    c["Cscf"] = C1 / 16384.0
    c["Sscnf"] = -S1 / 16384.0
    return {n: np.ascontiguousarray(a, dtype=np.float32) for n, a in c.items()}


CONST_SHAPES = {
    "WMOV": [KS, 512],
    "ZC0": [KS, 128], "ZC1": [KS, 128],
    "ZS0": [KS, 128], "ZS1": [KS, 128],
    "ZSn0": [KS, 128], "ZSn1": [KS, 128],
    "CS": [128, 256], "CSn": [128, 256], "SnCn": [128, 256],
    "ICSf": [128, 256], "ISnCf": [128, 256],
    "Cscf": [128, 128], "Sscnf": [128, 128],
}


def build_nc(n_planes=NPL):
    nc = bacc.Bacc("TRN2", target_bir_lowering=False, debug=False, num_devices=N_CORES)

    xs_t = nc.dram_tensor("xs", [n_planes, 128, 128], F32R, kind="ExternalInput")
    kt_t = nc.dram_tensor("kt", [n_planes, KS, KS], F32R, kind="ExternalInput")
    be_t = nc.dram_tensor("besb", [128, n_planes], F32, kind="ExternalInput")
    const_t = {n: nc.dram_tensor(n, s, (F32 if n.endswith("f") else F32R), kind="ExternalInput") for n, s in CONST_SHAPES.items()}
    out_t = nc.dram_tensor("out", [n_planes, 4, 128, 128], F32, kind="ExternalOutput")

    X = mybir.AxisListType.X

    with tile.TileContext(nc) as tc:
        with (
            tc.tile_pool(name="cpool", bufs=1) as cpool,
            tc.tile_pool(name="small", bufs=3) as small,
            tc.tile_pool(name="med", bufs=2) as med,
            tc.tile_pool(name="psum", bufs=1, space="PSUM") as pp,
        ):
            cs = {}
            for n, s in CONST_SHAPES.items():
                cs[n] = cpool.tile(s, (F32 if n.endswith("f") else F32R), tag=n, name=f"c_{n}")
                nc.sync.dma_start(cs[n][:], const_t[n][:])
            besb = cpool.tile([128, n_planes], F32, tag="besb")
            nc.sync.dma_start(besb[:], be_t[:])

            # one-time bf16 conversions for the ifft constants
            ICS = cpool.tile([128, 256], BF16, tag="ICS")
            ISnC = cpool.tile([128, 256], BF16, tag="ISnC")
            Csc = cpool.tile([128, 128], BF16, tag="Csc")
            Sscn = cpool.tile([128, 128], BF16, tag="Sscn")
            nc.scalar.copy(ICS[:], cs["ICSf"][:])
            nc.scalar.copy(ISnC[:], cs["ISnCf"][:])
            nc.scalar.copy(Csc[:], cs["Cscf"][:])
            nc.scalar.copy(Sscn[:], cs["Sscnf"][:])

            for i in range(n_planes):
                # ---- A = kt^T-transform of psf columns ----
                kt_sb = small.tile([KS, KS], F32R, tag="kt_sb")
                nc.sync.dma_start(kt_sb[:], kt_t[i])
                Aps = pp.tile([KS, 512], F32, tag="Aps")
                nc.tensor.matmul(Aps[:], kt_sb[:], cs["WMOV"][:], start=True, stop=True)
                AX = med.tile([KS, 1024], F32R, tag="AX")
                nc.scalar.copy(AX[:, 0:512], Aps[:])
                nc.scalar.mul(AX[:, 512:1024], Aps[:], -1.0)
                AXv = AX[:].rearrange("c (b f) -> c b f", b=8)

                # ---- Khat per row-parity p: psum [Kr_p0|Kr_p1|Ks_p0|Ks_p1] ----
                Kps = []
                for p in (0, 1):
                    kp = pp.tile([128, 512], F32, tag="big512", bufs=2)
                    ZC, ZS = cs[f"ZC{p}"], cs[f"ZS{p}"]
                    nc.tensor.matmul(kp[:, 0:256], ZC[:], AX[:, 0:256], start=True, stop=False)
                    nc.tensor.matmul(kp[:, 0:256], ZS[:], AX[:, 768:1024], start=False, stop=True)
                    nc.tensor.matmul(kp[:, 256:512], ZC[:], AX[:, 256:512], start=True, stop=False)
                    nc.tensor.matmul(kp[:, 256:512], ZS[:], AX[:, 0:256], start=False, stop=True)
                    Kps.append(kp)

                # ---- Y0' = sum_pq Khat_pq via psum accumulation ----
                Y0ps = pp.tile([128, 256], F32, tag="Y0ps")
                first = True
                for p in (0, 1):
                    ZC, ZSn = cs[f"ZC{p}"], cs[f"ZSn{p}"]
                    for q in (0, 1):
                        mv1 = AXv[:, q:q + 4:2, :]          # [Ar_q | As_q]
                        mv2 = AXv[:, 2 + q:6 + q:2, :]      # [As_q | Arn_q]
                        nc.tensor.matmul(Y0ps[:], ZC[:], mv1, start=first, stop=False)
                        last = (p == 1 and q == 1)
                        nc.tensor.matmul(Y0ps[:], ZSn[:], mv2, start=False, stop=last)
                        first = False

                # ---- T_p (bf16 Khat) and W' ----
                T = []
                for p in (0, 1):
                    tp = med.tile([128, 512], BF16, tag=f"T{p}")
                    nc.scalar.copy(tp[:], Kps[p][:])
                    T.append(tp)
                sq0 = med.tile([128, 512], F32, tag="sq0")
                sq1 = med.tile([128, 512], F32, tag="sq1")
                nc.scalar.square(sq0[:], Kps[0][:])
                nc.scalar.square(sq1[:], Kps[1][:])
                sqs = med.tile([128, 512], F32, tag="sqs")
                nc.gpsimd.tensor_add(sqs[:], sq0[:], sq1[:])
                Wt = small.tile([128, 128], F32, tag="Wt")
                nc.vector.tensor_reduce(Wt[:], sqs[:].rearrange("p (b f) -> p f b", b=4), X, ADD)

                # ---- wt = (1 - Y0')/(W' + be) ----
                den = small.tile([128, 128], F32, tag="den")
                nc.vector.tensor_scalar_add(den[:], Wt[:], besb[:, i:i + 1])
                dinv = small.tile([128, 128], F32, tag="dinv")
                nc.vector.reciprocal_approx_fast(dinv[:], den[:])
                num = small.tile([128, 128], F32, tag="num")
                nc.scalar.activation(num[:], Y0ps[:, 0:128], ACT_COPY, bias=1.0, scale=-1.0)
                wr = small.tile([128, 128], BF16, tag="wr")
                nc.vector.tensor_mul(wr[:], num[:], dinv[:])
                ws = small.tile([128, 128], BF16, tag="ws")
                nc.vector.scalar_tensor_tensor(ws[:], Y0ps[:, 128:256], -1.0, dinv[:], MULT, MULT)

                # ---- g = DFT2_128(x) ----
                x_sb = small.tile([128, 128], F32R, tag="x_sb")
                nc.sync.dma_start(x_sb[:], xs_t[i])
                Pps = pp.tile([128, 256], F32, tag="Pps")
                nc.tensor.matmul(Pps[:], x_sb[:], cs["CS"][:], start=True, stop=True)
                Pcp = small.tile([128, 256], F32R, tag="Pcp")
                nc.scalar.copy(Pcp[:], Pps[:])
                gps = pp.tile([128, 256], F32, tag="gps")
                nc.tensor.matmul(gps[:], Pcp[:, 0:128], cs["CSn"][:], start=True, stop=False)
                nc.tensor.matmul(gps[:], Pcp[:, 128:256], cs["SnCn"][:], start=False, stop=True)
                gb = small.tile([128, 256], BF16, tag="gb")
                nc.scalar.copy(gb[:], gps[:])

                # ---- v = g * wt (bf16) ----
                t1 = small.tile([128, 128], BF16, tag="t1")
                t2 = small.tile([128, 128], BF16, tag="t2")
                nc.vector.tensor_mul(t1[:], gb[:, 0:128], wr[:])
                nc.vector.tensor_mul(t2[:], gb[:, 128:256], ws[:])
                vr = small.tile([128, 128], BF16, tag="vr")
                nc.vector.tensor_sub(vr[:], t1[:], t2[:])
                t3 = small.tile([128, 128], BF16, tag="t3")
                t4 = small.tile([128, 128], BF16, tag="t4")
                nc.vector.tensor_mul(t3[:], gb[:, 0:128], ws[:])
                nc.vector.tensor_mul(t4[:], gb[:, 128:256], wr[:])
                vs = small.tile([128, 128], BF16, tag="vs")
                nc.vector.tensor_add(vs[:], t3[:], t4[:])
                vr2 = vr[:].unsqueeze(1).broadcast_to([128, 2, 128])
                vs2 = vs[:].unsqueeze(1).broadcast_to([128, 2, 128])

                # ---- M_pq = conj(Khat_pq) * v ; ifft + x ; store ----
                out_sb = med.tile([128, 512], F32, tag="out_sb")
                R2ps = pp.tile([128, 512], F32, tag="R2ps")
                for p in (0, 1):
                    KRv = T[p][:, 0:256].rearrange("p (b f) -> p b f", b=2)
                    KSv = T[p][:, 256:512].rearrange("p (b f) -> p b f", b=2)
                    m1 = small.tile([128, 256], BF16, tag="m1")
                    m2 = small.tile([128, 256], BF16, tag="m2")
                    MR = med.tile([128, 256], BF16, tag=f"MR{p}")
                    b2 = lambda ap: ap.rearrange("p (b f) -> p b f", b=2)  # noqa: E731
                    nc.vector.tensor_mul(b2(m1[:]), KRv, vr2)
                    nc.vector.tensor_mul(b2(m2[:]), KSv, vs2)
                    nc.vector.tensor_add(MR[:], m1[:], m2[:])
                    m3 = small.tile([128, 256], BF16, tag="m3")
                    m4 = small.tile([128, 256], BF16, tag="m4")
                    MS = med.tile([128, 256], BF16, tag=f"MS{p}")
                    nc.vector.tensor_mul(b2(m3[:]), KRv, vs2)
                    nc.vector.tensor_mul(b2(m4[:]), KSv, vr2)
                    nc.vector.tensor_sub(MS[:], m3[:], m4[:])

                    R1ps = pp.tile([128, 512], F32, tag="big512", bufs=2)
                    for q in (0, 1):
                        rsl = slice(q * 256, (q + 1) * 256)
                        qsl = slice(q * 128, (q + 1) * 128)
                        nc.tensor.matmul(R1ps[:, rsl], MR[:, qsl], ICS[:], start=True, stop=False)
                        nc.tensor.matmul(R1ps[:, rsl], MS[:, qsl], ISnC[:], start=False, stop=True)
                        R1cp = small.tile([128, 256], BF16, tag="R1cp")
                        if q == 0:
                            nc.vector.tensor_copy(R1cp[:], R1ps[:, rsl])
                        else:
                            nc.scalar.copy(R1cp[:], R1ps[:, rsl])
                        blk = 2 * p + q
                        bsl = slice(blk * 128, (blk + 1) * 128)
                        nc.tensor.matmul(R2ps[:, bsl], R1cp[:, 0:128], Csc[:], start=True, stop=False)
                        nc.tensor.matmul(R2ps[:, bsl], R1cp[:, 128:256], Sscn[:], start=False, stop=True)
                        nc.vector.tensor_add(out_sb[:, bsl], R2ps[:, bsl], x_sb[:].bitcast(F32))

                nc.sync.dma_start(
                    out_t[i].rearrange("pq m f -> m pq f"),
                    out_sb[:].rearrange("p (pq f) -> p pq f", pq=4),
                )

    nc.compile()
    return nc


@functools.lru_cache(maxsize=2)
def _built(n_planes=NPL):
    return build_nc(n_planes)


def make_in_maps(x, k, alpha, n_planes=NPL, n_cores=N_CORES):
    consts = _host_consts()
    alpha_c = alpha.reshape(-1).astype(np.float64)  # [64]
    be = (1.0 / (1.0 + np.exp(-(alpha_c - 9.0))) + 1e-3).astype(np.float32)
    cpc = n_planes // 4  # channels per core
    in_maps = []
    for ci in range(n_cores):
        chs = slice(ci * cpc, (ci + 1) * cpc)
        xs = np.ascontiguousarray(x[:, chs].transpose(1, 0, 2, 3).reshape(n_planes, 128, 128))
        kt = np.ascontiguousarray(k[:, chs].transpose(1, 0, 3, 2).reshape(n_planes, KS, KS))
        be_pl = np.repeat(be[chs], 4)  # plane order: (c_loc, b)
        besb = np.broadcast_to(be_pl, (128, n_planes)).astype(np.float32).copy()
        m = {"xs": xs, "kt": kt, "besb": besb}
        m.update(consts)
        in_maps.append(m)
    return in_maps


def _assemble(res, n_planes=NPL, n_cores=N_CORES):
    out = np.empty((4, 64, 256, 256), np.float32)
    cpc = n_planes // 4
    for ci in range(n_cores):
        o = res.results[ci]["out"].reshape(cpc, 4, 2, 2, 128, 128)
        # [c_loc, b, p, q, m, n] -> [b, c_loc, m, p, n, q] -> [b, c_loc, 256, 256]
        o = o.transpose(1, 0, 4, 2, 5, 3).reshape(4, cpc, 256, 256)
        out[:, ci * cpc:(ci + 1) * cpc] = o
    return out


def kernel(x, k, alpha, sf=2, **_ignored):
    x = np.asarray(x, dtype=np.float32)
    k = np.asarray(k, dtype=np.float32)
    alpha = np.asarray(alpha, dtype=np.float32)
    assert int(sf) == 2 and x.shape == (4, 64, 128, 128) and k.shape == (4, 64, KS, KS)

    nc = _built(NPL)
    in_maps = make_in_maps(x, k, alpha)
    res = run_bass_kernel_spmd(nc, in_maps, core_ids=list(range(N_CORES)))
    return _assemble(res)


if __name__ == "__main__":
    rng = np.random.default_rng(0)
    x = rng.standard_normal((4, 64, 128, 128), dtype=np.float32)
    k = rng.random((4, 64, KS, KS), dtype=np.float32)
    alpha = np.zeros((1, 64, 1, 1), np.float32)
    out = kernel(x, k, alpha, 2)
    print("out", out.shape, out.dtype, float(np.abs(out).max()))


# revision 13
# speedup vs baseline: 1.9089x; 1.4342x over previous
"""Trainium2 Bass kernel for ConvReverseDataNet (USRNet-style FFT data step).

128-grid reformulation (per (b,c) plane, sf=2), validated in fp64/bf16 numpy:
  g   = DFT2_128(x)
  s_pq[a,b] = rolled_psf256[2a+p, 2b+q]   (parity subkernels of the 25x25 psf)
  Khat_pq = DFT2_128(s_pq) * e^{-2pi i(u p + v q)/128}  (twiddles folded into
            the small-DFT constants via positions (j+par)/2)
  W'  = sum_pq |Khat_pq|^2 ;  Y0' = sum_pq Khat_pq      (both on 128 grid)
  wt  = (1 - Y0') / (W' + be) ;  v = g * wt
  out[2m+p, 2n+q] = real(IDFT2_128(conj(Khat_pq) * v))[m,n] + x[m,n]

mm(stat, mov) = stat.T @ mov (contract over partition dim). Complex arrays
X stored as adjacent [Xr | Xs] blocks. Big-DFT matmuls run in float32r
(1 cycle/row at N>=256); the M/ifft stages run in bf16 (validated 3.2e-3).

Sharding: 256 (b,c) planes over 8 cores; core ci gets channels ci*8..ci*8+7.
Output is parity-planar [plane, 2p+q, m, n]; host interleaves.
"""

import functools
import sys

import numpy as np

if "/opt/trn_rl_repo" not in sys.path:
    sys.path.insert(0, "/opt/trn_rl_repo")

from concourse import bacc, bass, mybir, tile  # noqa: E402
from concourse.bass_utils import run_bass_kernel_spmd  # noqa: E402

F32 = mybir.dt.float32
F32R = mybir.dt.float32r
BF16 = mybir.dt.bfloat16
MULT = mybir.AluOpType.mult
ADD = mybir.AluOpType.add
ACT_COPY = mybir.ActivationFunctionType.Copy

N_CORES = 8
NPL = 32  # planes per core
KS = 25


def _host_consts():
    t = np.arange(KS)
    j = (t - 12) % 256            # position in rolled 256 grid
    par = t % 2                   # parity (row p / col q)
    bp = (j + par) // 2           # twiddle-folded 128-grid position

    u = np.arange(128)
    ang = 2 * np.pi * np.outer(bp, u) / 128  # [25,128]
    COSg = np.cos(ang).astype(np.float32)
    SINg = np.sin(ang).astype(np.float32)
    sel = [(par == 0).astype(np.float32)[:, None], (par == 1).astype(np.float32)[:, None]]

    c = {}
    c["WMOV"] = np.concatenate(
        [COSg * sel[0], COSg * sel[1], -SINg * sel[0], -SINg * sel[1]], axis=1
    )  # [25,512] -> A blocks [Ar0|Ar1|As0|As1]
    for p in (0, 1):
        c[f"ZC{p}"] = COSg * sel[p]
        c[f"ZS{p}"] = -SINg * sel[p]
        c[f"ZSn{p}"] = SINg * sel[p]

    th = 2 * np.pi * np.outer(u, u) / 128
    C1 = np.cos(th).astype(np.float32)
    S1 = np.sin(th).astype(np.float32)
    c["CS"] = np.concatenate([C1, S1], 1)
    c["CSn"] = np.concatenate([C1, -S1], 1)
    c["SnCn"] = np.concatenate([-S1, -C1], 1)
    c["ICSf"] = np.concatenate([C1, S1], 1)
    c["ISnCf"] = np.concatenate([-S1, C1], 1)
    c["Cscf"] = C1 / 16384.0
    c["Sscnf"] = -S1 / 16384.0
    return {n: np.ascontiguousarray(a, dtype=np.float32) for n, a in c.items()}


CONST_SHAPES = {
    "WMOV": [KS, 512],
    "ZC0": [KS, 128], "ZC1": [KS, 128],
    "ZS0": [KS, 128], "ZS1": [KS, 128],
    "ZSn0": [KS, 128], "ZSn1": [KS, 128],
    "CS": [128, 256], "CSn": [128, 256], "SnCn": [128, 256],
    "ICSf": [128, 256], "ISnCf": [128, 256],
    "Cscf": [128, 128], "Sscnf": [128, 128],
}


def build_nc(n_planes=NPL):
    nc = bacc.Bacc("TRN2", target_bir_lowering=False, debug=False, num_devices=N_CORES)

    xs_t = nc.dram_tensor("xs", [n_planes, 128, 128], F32R, kind="ExternalInput")
    kt_t = nc.dram_tensor("kt", [n_planes, KS, KS], F32R, kind="ExternalInput")
    be_t = nc.dram_tensor("besb", [128, n_planes], F32, kind="ExternalInput")
    const_t = {n: nc.dram_tensor(n, s, (F32 if n.endswith("f") else F32R), kind="ExternalInput") for n, s in CONST_SHAPES.items()}
    out_t = nc.dram_tensor("out", [n_planes, 4, 128, 128], F32, kind="ExternalOutput")

    X = mybir.AxisListType.X

    with tile.TileContext(nc) as tc:
        with (
            tc.tile_pool(name="cpool", bufs=1) as cpool,
            tc.tile_pool(name="small", bufs=3) as small,
            tc.tile_pool(name="med", bufs=2) as med,
            tc.tile_pool(name="psum", bufs=1, space="PSUM") as pp,
        ):
            cs = {}
            for n, s in CONST_SHAPES.items():
                cs[n] = cpool.tile(s, (F32 if n.endswith("f") else F32R), tag=n, name=f"c_{n}")
                nc.sync.dma_start(cs[n][:], const_t[n][:])
            besb = cpool.tile([128, n_planes], F32, tag="besb")
            nc.sync.dma_start(besb[:], be_t[:])

            # one-time bf16 conversions for the ifft constants
            ICS = cpool.tile([128, 256], BF16, tag="ICS")
            ISnC = cpool.tile([128, 256], BF16, tag="ISnC")
            Csc = cpool.tile([128, 128], BF16, tag="Csc")
            Sscn = cpool.tile([128, 128], BF16, tag="Sscn")
            nc.scalar.copy(ICS[:], cs["ICSf"][:])
            nc.scalar.copy(ISnC[:], cs["ISnCf"][:])
            nc.scalar.copy(Csc[:], cs["Cscf"][:])
            nc.scalar.copy(Sscn[:], cs["Sscnf"][:])

            for i in range(n_planes):
                # ---- A = kt^T-transform of psf columns ----
                kt_sb = small.tile([KS, KS], F32R, tag="kt_sb")
                nc.sync.dma_start(kt_sb[:], kt_t[i])
                Aps = pp.tile([KS, 512], F32, tag="Aps")
                nc.tensor.matmul(Aps[:], kt_sb[:], cs["WMOV"][:], start=True, stop=True)
                AX = med.tile([KS, 1024], F32R, tag="AX")
                nc.scalar.copy(AX[:, 0:512], Aps[:])
                nc.scalar.mul(AX[:, 512:1024], Aps[:], -1.0)
                AXv = AX[:].rearrange("c (b f) -> c b f", b=8)

                # ---- Khat per row-parity p: psum [Kr_p0|Kr_p1|Ks_p0|Ks_p1] ----
                Kps = []
                for p in (0, 1):
                    kp = pp.tile([128, 512], F32, tag="big512", bufs=2)
                    ZC, ZS = cs[f"ZC{p}"], cs[f"ZS{p}"]
                    nc.tensor.matmul(kp[:, 0:256], ZC[:], AX[:, 0:256], start=True, stop=False)
                    nc.tensor.matmul(kp[:, 0:256], ZS[:], AX[:, 768:1024], start=False, stop=True)
                    nc.tensor.matmul(kp[:, 256:512], ZC[:], AX[:, 256:512], start=True, stop=False)
                    nc.tensor.matmul(kp[:, 256:512], ZS[:], AX[:, 0:256], start=False, stop=True)
                    Kps.append(kp)

                # ---- Y0' = sum_pq Khat_pq via psum accumulation ----
                Y0ps = pp.tile([128, 256], F32, tag="Y0ps")
                first = True
                for p in (0, 1):
                    ZC, ZSn = cs[f"ZC{p}"], cs[f"ZSn{p}"]
                    for q in (0, 1):
                        mv1 = AXv[:, q:q + 4:2, :]          # [Ar_q | As_q]
                        mv2 = AXv[:, 2 + q:6 + q:2, :]      # [As_q | Arn_q]
                        nc.tensor.matmul(Y0ps[:], ZC[:], mv1, start=first, stop=False)
                        last = (p == 1 and q == 1)
                        nc.tensor.matmul(Y0ps[:], ZSn[:], mv2, start=False, stop=last)
                        first = False

                # ---- T_p (bf16 Khat) and W' ----
                T = []
                for p in (0, 1):
                    tp = med.tile([128, 512], BF16, tag=f"T{p}")
                    nc.scalar.copy(tp[:], Kps[p][:])
                    T.append(tp)
                sq0 = med.tile([128, 512], F32, tag="sq0")
                sq1 = med.tile([128, 512], F32, tag="sq1")
                nc.scalar.square(sq0[:], Kps[0][:])
                nc.scalar.square(sq1[:], Kps[1][:])
                sqs = med.tile([128, 512], F32, tag="sqs")
                nc.vector.tensor_add(sqs[:], sq0[:], sq1[:])
                sqh = small.tile([128, 256], F32, tag="sqh")
                nc.vector.tensor_add(sqh[:], sqs[:, 0:256], sqs[:, 256:512])
                Wt = small.tile([128, 128], F32, tag="Wt")
                nc.vector.tensor_add(Wt[:], sqh[:, 0:128], sqh[:, 128:256])

                # ---- wt = (1 - Y0')/(W' + be) ----
                den = small.tile([128, 128], F32, tag="den")
                nc.vector.tensor_scalar_add(den[:], Wt[:], besb[:, i:i + 1])
                dinv = small.tile([128, 128], F32, tag="dinv")
                nc.vector.reciprocal_approx_fast(dinv[:], den[:])
                num = small.tile([128, 128], F32, tag="num")
                nc.scalar.activation(num[:], Y0ps[:, 0:128], ACT_COPY, bias=1.0, scale=-1.0)
                wr = small.tile([128, 128], BF16, tag="wr")
                nc.vector.tensor_mul(wr[:], num[:], dinv[:])
                ws = small.tile([128, 128], BF16, tag="ws")
                nc.vector.scalar_tensor_tensor(ws[:], Y0ps[:, 128:256], -1.0, dinv[:], MULT, MULT)

                # ---- g = DFT2_128(x) ----
                x_sb = small.tile([128, 128], F32R, tag="x_sb")
                nc.sync.dma_start(x_sb[:], xs_t[i])
                Pps = pp.tile([128, 256], F32, tag="Pps")
                nc.tensor.matmul(Pps[:], x_sb[:], cs["CS"][:], start=True, stop=True)
                Pcp = small.tile([128, 256], F32R, tag="Pcp")
                nc.scalar.copy(Pcp[:], Pps[:])
                gps = pp.tile([128, 256], F32, tag="gps")
                nc.tensor.matmul(gps[:], Pcp[:, 0:128], cs["CSn"][:], start=True, stop=False)
                nc.tensor.matmul(gps[:], Pcp[:, 128:256], cs["SnCn"][:], start=False, stop=True)
                gb = small.tile([128, 256], BF16, tag="gb")
                nc.scalar.copy(gb[:], gps[:])

                # ---- v = g * wt (bf16) ----
                t1 = small.tile([128, 128], BF16, tag="t1")
                t2 = small.tile([128, 128], BF16, tag="t2")
                nc.vector.tensor_mul(t1[:], gb[:, 0:128], wr[:])
                nc.vector.tensor_mul(t2[:], gb[:, 128:256], ws[:])
                vr = small.tile([128, 128], BF16, tag="vr")
                nc.vector.tensor_sub(vr[:], t1[:], t2[:])
                t3 = small.tile([128, 128], BF16, tag="t3")
                t4 = small.tile([128, 128], BF16, tag="t4")
                nc.vector.tensor_mul(t3[:], gb[:, 0:128], ws[:])
                nc.vector.tensor_mul(t4[:], gb[:, 128:256], wr[:])
                vs = small.tile([128, 128], BF16, tag="vs")
                nc.vector.tensor_add(vs[:], t3[:], t4[:])
                vr2 = vr[:].unsqueeze(1).broadcast_to([128, 2, 128])
                vs2 = vs[:].unsqueeze(1).broadcast_to([128, 2, 128])

                # ---- M_pq = conj(Khat_pq) * v ; ifft + x ; store ----
                out_sb = med.tile([128, 512], F32, tag="out_sb")
                R2ps = pp.tile([128, 512], F32, tag="R2ps")
                for p in (0, 1):
                    KRv = T[p][:, 0:256].rearrange("p (b f) -> p b f", b=2)
                    KSv = T[p][:, 256:512].rearrange("p (b f) -> p b f", b=2)
                    m1 = small.tile([128, 256], BF16, tag="m1")
                    m2 = small.tile([128, 256], BF16, tag="m2")
                    MR = med.tile([128, 256], BF16, tag=f"MR{p}")
                    b2 = lambda ap: ap.rearrange("p (b f) -> p b f", b=2)  # noqa: E731
                    nc.vector.tensor_mul(b2(m1[:]), KRv, vr2)
                    nc.vector.tensor_mul(b2(m2[:]), KSv, vs2)
                    nc.vector.tensor_add(MR[:], m1[:], m2[:])
                    m3 = small.tile([128, 256], BF16, tag="m3")
                    m4 = small.tile([128, 256], BF16, tag="m4")
                    MS = med.tile([128, 256], BF16, tag=f"MS{p}")
                    nc.vector.tensor_mul(b2(m3[:]), KRv, vs2)
                    nc.vector.tensor_mul(b2(m4[:]), KSv, vr2)
                    nc.vector.tensor_sub(MS[:], m3[:], m4[:])

                    R1ps = pp.tile([128, 512], F32, tag="R1ps")
                    for q in (0, 1):
                        rsl = slice(q * 256, (q + 1) * 256)
                        qsl = slice(q * 128, (q + 1) * 128)
                        nc.tensor.matmul(R1ps[:, rsl], MR[:, qsl], ICS[:], start=True, stop=False)
                        nc.tensor.matmul(R1ps[:, rsl], MS[:, qsl], ISnC[:], start=False, stop=True)
                        R1cp = small.tile([128, 256], BF16, tag="R1cp")
                        if q == 0:
                            nc.vector.tensor_copy(R1cp[:], R1ps[:, rsl])
                        else:
                            nc.scalar.copy(R1cp[:], R1ps[:, rsl])
                        blk = 2 * p + q
                        bsl = slice(blk * 128, (blk + 1) * 128)
                        nc.tensor.matmul(R2ps[:, bsl], R1cp[:, 0:128], Csc[:], start=True, stop=False)
                        nc.tensor.matmul(R2ps[:, bsl], R1cp[:, 128:256], Sscn[:], start=False, stop=True)
                        nc.vector.tensor_add(out_sb[:, bsl], R2ps[:, bsl], x_sb[:].bitcast(F32))

                nc.sync.dma_start(
                    out_t[i].rearrange("pq m f -> m pq f"),
                    out_sb[:].rearrange("p (pq f) -> p pq f", pq=4),
                )

    nc.compile()
    return nc


@functools.lru_cache(maxsize=2)
def _built(n_planes=NPL):
    return build_nc(n_planes)


def make_in_maps(x, k, alpha, n_planes=NPL, n_cores=N_CORES):
    consts = _host_consts()
    alpha_c = alpha.reshape(-1).astype(np.float64)  # [64]
    be = (1.0 / (1.0 + np.exp(-(alpha_c - 9.0))) + 1e-3).astype(np.float32)
    cpc = n_planes // 4  # channels per core
    in_maps = []
    for ci in range(n_cores):
        chs = slice(ci * cpc, (ci + 1) * cpc)
        xs = np.ascontiguousarray(x[:, chs].transpose(1, 0, 2, 3).reshape(n_planes, 128, 128))
        kt = np.ascontiguousarray(k[:, chs].transpose(1, 0, 3, 2).reshape(n_planes, KS, KS))
        be_pl = np.repeat(be[chs], 4)  # plane order: (c_loc, b)
        besb = np.broadcast_to(be_pl, (128, n_planes)).astype(np.float32).copy()
        m = {"xs": xs, "kt": kt, "besb": besb}
        m.update(consts)
        in_maps.append(m)
    return in_maps


def _assemble(res, n_planes=NPL, n_cores=N_CORES):
    out = np.empty((4, 64, 256, 256), np.float32)
    cpc = n_planes // 4
    for ci in range(n_cores):
        o = res.results[ci]["out"].reshape(cpc, 4, 2, 2, 128, 128)
        # [c_loc, b, p, q, m, n] -> [b, c_loc, m, p, n, q] -> [b, c_loc, 256, 256]
        o = o.transpose(1, 0, 4, 2, 5, 3).reshape(4, cpc, 256, 256)
        out[:, ci * cpc:(ci + 1) * cpc] = o
    return out


def kernel(x, k, alpha, sf=2, **_ignored):
    x = np.asarray(x, dtype=np.float32)
    k = np.asarray(k, dtype=np.float32)
    alpha = np.asarray(alpha, dtype=np.float32)
    assert int(sf) == 2 and x.shape == (4, 64, 128, 128) and k.shape == (4, 64, KS, KS)

    nc = _built(NPL)
    in_maps = make_in_maps(x, k, alpha)
    res = run_bass_kernel_spmd(nc, in_maps, core_ids=list(range(N_CORES)))
    return _assemble(res)


if __name__ == "__main__":
    rng = np.random.default_rng(0)
    x = rng.standard_normal((4, 64, 128, 128), dtype=np.float32)
    k = rng.random((4, 64, KS, KS), dtype=np.float32)
    alpha = np.zeros((1, 64, 1, 1), np.float32)
    out = kernel(x, k, alpha, 2)
    print("out", out.shape, out.dtype, float(np.abs(out).max()))


# revision 17
# speedup vs baseline: 2.0623x; 1.0803x over previous
"""Trainium2 Bass kernel for ConvReverseDataNet (USRNet-style FFT data step).

128-grid reformulation (per (b,c) plane, sf=2), validated in fp64/bf16 numpy:
  g   = DFT2_128(x)
  s_pq[a,b] = rolled_psf256[2a+p, 2b+q]   (parity subkernels of the 25x25 psf)
  Khat_pq = DFT2_128(s_pq) * e^{-2pi i(u p + v q)/128}  (twiddles folded into
            the small-DFT constants via positions (j+par)/2)
  W'  = sum_pq |Khat_pq|^2 ;  Y0' = sum_pq Khat_pq      (both on 128 grid)
  wt  = (1 - Y0') / (W' + be) ;  v = g * wt
  out[2m+p, 2n+q] = real(IDFT2_128(conj(Khat_pq) * v))[m,n] + x[m,n]

mm(stat, mov) = stat.T @ mov (contract over partition dim). Complex arrays
X stored as adjacent [Xr | Xs] blocks. Big-DFT matmuls run in float32r
(1 cycle/row at N>=256); the M/ifft stages run in bf16 (validated 3.2e-3).

Sharding: 256 (b,c) planes over 8 cores; core ci gets channels ci*8..ci*8+7.
Output is parity-planar [plane, 2p+q, m, n]; host interleaves.
"""

import functools
import sys

import numpy as np

if "/opt/trn_rl_repo" not in sys.path:
    sys.path.insert(0, "/opt/trn_rl_repo")

from concourse import bacc, bass, mybir, tile  # noqa: E402
from concourse.bass_utils import run_bass_kernel_spmd  # noqa: E402

F32 = mybir.dt.float32
F32R = mybir.dt.float32r
BF16 = mybir.dt.bfloat16
MULT = mybir.AluOpType.mult
ADD = mybir.AluOpType.add
ACT_COPY = mybir.ActivationFunctionType.Copy

N_CORES = 8
NPL = 32  # planes per core
KS = 25


def _host_consts():
    t = np.arange(KS)
    j = (t - 12) % 256            # position in rolled 256 grid
    par = t % 2                   # parity (row p / col q)
    bp = (j + par) // 2           # twiddle-folded 128-grid position

    u = np.arange(128)
    ang = 2 * np.pi * np.outer(bp, u) / 128  # [25,128]
    COSg = np.cos(ang).astype(np.float32)
    SINg = np.sin(ang).astype(np.float32)
    sel = [(par == 0).astype(np.float32)[:, None], (par == 1).astype(np.float32)[:, None]]

    c = {}
    c["WMOV"] = np.concatenate(
        [COSg * sel[0], COSg * sel[1], -SINg * sel[0], -SINg * sel[1]], axis=1
    )  # [25,512] -> A blocks [Ar0|Ar1|As0|As1]
    for p in (0, 1):
        c[f"ZC{p}"] = COSg * sel[p]
        c[f"ZS{p}"] = -SINg * sel[p]
        c[f"ZSn{p}"] = SINg * sel[p]

    th = 2 * np.pi * np.outer(u, u) / 128
    C1 = np.cos(th).astype(np.float32)
    S1 = np.sin(th).astype(np.float32)
    c["CS"] = np.concatenate([C1, S1], 1)
    c["CSn"] = np.concatenate([C1, -S1], 1)
    c["SnCn"] = np.concatenate([-S1, -C1], 1)
    c["ICSf"] = np.concatenate([C1, S1], 1)
    c["ISnCf"] = np.concatenate([-S1, C1], 1)
    c["Cscf"] = C1 / 16384.0
    c["Sscnf"] = -S1 / 16384.0
    return {n: np.ascontiguousarray(a, dtype=np.float32) for n, a in c.items()}


CONST_SHAPES = {
    "WMOV": [KS, 512],
    "ZC0": [KS, 128], "ZC1": [KS, 128],
    "ZS0": [KS, 128], "ZS1": [KS, 128],
    "ZSn0": [KS, 128], "ZSn1": [KS, 128],
    "CS": [128, 256], "CSn": [128, 256], "SnCn": [128, 256],
    "ICSf": [128, 256], "ISnCf": [128, 256],
    "Cscf": [128, 128], "Sscnf": [128, 128],
}


def build_nc(n_planes=NPL):
    nc = bacc.Bacc("TRN2", target_bir_lowering=False, debug=False, num_devices=N_CORES)

    xs_t = nc.dram_tensor("xs", [n_planes, 128, 128], F32R, kind="ExternalInput")
    kt_t = nc.dram_tensor("kt", [n_planes, KS, KS], F32R, kind="ExternalInput")
    be_t = nc.dram_tensor("besb", [128, n_planes], F32, kind="ExternalInput")
    const_t = {n: nc.dram_tensor(n, s, (F32 if n.endswith("f") else F32R), kind="ExternalInput") for n, s in CONST_SHAPES.items()}
    out_t = nc.dram_tensor("out", [n_planes, 4, 128, 128], F32, kind="ExternalOutput")

    X = mybir.AxisListType.X

    with tile.TileContext(nc) as tc:
        with (
            tc.tile_pool(name="cpool", bufs=1) as cpool,
            tc.tile_pool(name="small", bufs=3) as small,
            tc.tile_pool(name="med", bufs=2) as med,
            tc.tile_pool(name="psum", bufs=1, space="PSUM") as pp,
        ):
            cs = {}
            for n, s in CONST_SHAPES.items():
                cs[n] = cpool.tile(s, (F32 if n.endswith("f") else F32R), tag=n, name=f"c_{n}")
                nc.sync.dma_start(cs[n][:], const_t[n][:])
            besb = cpool.tile([128, n_planes], F32, tag="besb")
            nc.sync.dma_start(besb[:], be_t[:])

            # one-time bf16 conversions for the ifft constants
            ICS = cpool.tile([128, 256], BF16, tag="ICS")
            ISnC = cpool.tile([128, 256], BF16, tag="ISnC")
            Csc = cpool.tile([128, 128], BF16, tag="Csc")
            Sscn = cpool.tile([128, 128], BF16, tag="Sscn")
            nc.scalar.copy(ICS[:], cs["ICSf"][:])
            nc.scalar.copy(ISnC[:], cs["ISnCf"][:])
            nc.scalar.copy(Csc[:], cs["Cscf"][:])
            nc.scalar.copy(Sscn[:], cs["Sscnf"][:])

            for i in range(n_planes):
                # ---- A = kt^T-transform of psf columns ----
                kt_sb = small.tile([KS, KS], F32R, tag="kt_sb")
                nc.sync.dma_start(kt_sb[:], kt_t[i])
                Aps = pp.tile([KS, 512], F32, tag="Aps")
                nc.tensor.matmul(Aps[:], kt_sb[:], cs["WMOV"][:], start=True, stop=True)
                AX = med.tile([KS, 1024], F32R, tag="AX")
                nc.scalar.copy(AX[:, 0:512], Aps[:])
                nc.scalar.mul(AX[:, 512:1024], Aps[:], -1.0)
                AXv = AX[:].rearrange("c (b f) -> c b f", b=8)

                # ---- Khat per row-parity p: psum [Kr_p0|Kr_p1|Ks_p0|Ks_p1] ----
                Kps = []
                for p in (0, 1):
                    kp = pp.tile([128, 512], F32, tag="big512", bufs=2)
                    ZC, ZS = cs[f"ZC{p}"], cs[f"ZS{p}"]
                    nc.tensor.matmul(kp[:, 0:256], ZC[:], AX[:, 0:256], start=True, stop=False)
                    nc.tensor.matmul(kp[:, 0:256], ZS[:], AX[:, 768:1024], start=False, stop=True)
                    nc.tensor.matmul(kp[:, 256:512], ZC[:], AX[:, 256:512], start=True, stop=False)
                    nc.tensor.matmul(kp[:, 256:512], ZS[:], AX[:, 0:256], start=False, stop=True)
                    Kps.append(kp)

                # ---- T_p (bf16 Khat) ----
                T = []
                for p in (0, 1):
                    tp = med.tile([128, 512], BF16, tag=f"T{p}")
                    nc.scalar.copy(tp[:], Kps[p][:])
                    T.append(tp)

                # ---- Y0' = sum_pq Khat_pq via gpsimd folds of T_p ----
                y0a = small.tile([128, 256], F32, tag="y0a")
                y0b = small.tile([128, 256], F32, tag="y0b")
                Y0f = small.tile([128, 256], F32, tag="Y0f")  # [Y0r | Y0s]
                hq = lambda ap: ap.rearrange("p (h q f) -> p h q f", h=2, q=2)  # noqa: E731
                hf = lambda ap: ap.rearrange("p (h f) -> p h f", h=2)  # noqa: E731
                nc.gpsimd.tensor_add(hf(y0a[:]), hq(T[0][:])[:, :, 0, :], hq(T[0][:])[:, :, 1, :])
                nc.gpsimd.tensor_add(hf(y0b[:]), hq(T[1][:])[:, :, 0, :], hq(T[1][:])[:, :, 1, :])
                nc.gpsimd.tensor_add(Y0f[:], y0a[:], y0b[:])
                sq0 = med.tile([128, 512], F32, tag="sq0")
                sq1 = med.tile([128, 512], F32, tag="sq1")
                nc.scalar.square(sq0[:], Kps[0][:])
                nc.scalar.square(sq1[:], Kps[1][:])
                sqs = med.tile([128, 512], F32, tag="sqs")
                nc.vector.tensor_add(sqs[:], sq0[:], sq1[:])
                sqh = small.tile([128, 256], F32, tag="sqh")
                nc.vector.tensor_add(sqh[:], sqs[:, 0:256], sqs[:, 256:512])
                Wt = small.tile([128, 128], F32, tag="Wt")
                nc.vector.tensor_add(Wt[:], sqh[:, 0:128], sqh[:, 128:256])

                # ---- wt = (1 - Y0')/(W' + be) ----
                den = small.tile([128, 128], F32, tag="den")
                nc.vector.tensor_scalar_add(den[:], Wt[:], besb[:, i:i + 1])
                dinv = small.tile([128, 128], F32, tag="dinv")
                nc.vector.reciprocal_approx_fast(dinv[:], den[:])
                num = small.tile([128, 128], F32, tag="num")
                nc.scalar.activation(num[:], Y0f[:, 0:128], ACT_COPY, bias=1.0, scale=-1.0)
                wr = small.tile([128, 128], BF16, tag="wr")
                nc.vector.tensor_mul(wr[:], num[:], dinv[:])
                ws = small.tile([128, 128], BF16, tag="ws")
                nc.vector.scalar_tensor_tensor(ws[:], Y0f[:, 128:256], -1.0, dinv[:], MULT, MULT)

                # ---- g = DFT2_128(x) ----
                x_sb = small.tile([128, 128], F32R, tag="x_sb")
                nc.sync.dma_start(x_sb[:], xs_t[i])
                Pps = pp.tile([128, 256], F32, tag="Pps")
                nc.tensor.matmul(Pps[:], x_sb[:], cs["CS"][:], start=True, stop=True)
                Pcp = small.tile([128, 256], F32R, tag="Pcp")
                nc.scalar.copy(Pcp[:], Pps[:])
                gps = pp.tile([128, 256], F32, tag="gps")
                nc.tensor.matmul(gps[:], Pcp[:, 0:128], cs["CSn"][:], start=True, stop=False)
                nc.tensor.matmul(gps[:], Pcp[:, 128:256], cs["SnCn"][:], start=False, stop=True)
                gb = small.tile([128, 256], BF16, tag="gb")
                nc.scalar.copy(gb[:], gps[:])

                # ---- v = g * wt (bf16) ----
                t1 = small.tile([128, 128], BF16, tag="t1")
                t2 = small.tile([128, 128], BF16, tag="t2")
                nc.vector.tensor_mul(t1[:], gb[:, 0:128], wr[:])
                nc.vector.tensor_mul(t2[:], gb[:, 128:256], ws[:])
                vr = small.tile([128, 128], BF16, tag="vr")
                nc.vector.tensor_sub(vr[:], t1[:], t2[:])
                t3 = small.tile([128, 128], BF16, tag="t3")
                t4 = small.tile([128, 128], BF16, tag="t4")
                nc.vector.tensor_mul(t3[:], gb[:, 0:128], ws[:])
                nc.vector.tensor_mul(t4[:], gb[:, 128:256], wr[:])
                vs = small.tile([128, 128], BF16, tag="vs")
                nc.vector.tensor_add(vs[:], t3[:], t4[:])
                vr2 = vr[:].unsqueeze(1).broadcast_to([128, 2, 128])
                vs2 = vs[:].unsqueeze(1).broadcast_to([128, 2, 128])

                # ---- M_pq = conj(Khat_pq) * v ; ifft + x ; store ----
                out_sb = med.tile([128, 512], F32, tag="out_sb")
                R2ps = pp.tile([128, 512], F32, tag="R2ps")
                for p in (0, 1):
                    KRv = T[p][:, 0:256].rearrange("p (b f) -> p b f", b=2)
                    KSv = T[p][:, 256:512].rearrange("p (b f) -> p b f", b=2)
                    m1 = small.tile([128, 256], BF16, tag="m1")
                    m2 = small.tile([128, 256], BF16, tag="m2")
                    MR = med.tile([128, 256], BF16, tag=f"MR{p}")
                    b2 = lambda ap: ap.rearrange("p (b f) -> p b f", b=2)  # noqa: E731
                    nc.vector.tensor_mul(b2(m1[:]), KRv, vr2)
                    nc.vector.tensor_mul(b2(m2[:]), KSv, vs2)
                    nc.vector.tensor_add(MR[:], m1[:], m2[:])
                    m3 = small.tile([128, 256], BF16, tag="m3")
                    m4 = small.tile([128, 256], BF16, tag="m4")
                    MS = med.tile([128, 256], BF16, tag=f"MS{p}")
                    nc.vector.tensor_mul(b2(m3[:]), KRv, vs2)
                    nc.vector.tensor_mul(b2(m4[:]), KSv, vr2)
                    nc.vector.tensor_sub(MS[:], m3[:], m4[:])

                    R1ps = pp.tile([128, 512], F32, tag="R1ps", bufs=2)
                    for q in (0, 1):
                        rsl = slice(q * 256, (q + 1) * 256)
                        qsl = slice(q * 128, (q + 1) * 128)
                        nc.tensor.matmul(R1ps[:, rsl], MR[:, qsl], ICS[:], start=True, stop=False)
                        nc.tensor.matmul(R1ps[:, rsl], MS[:, qsl], ISnC[:], start=False, stop=True)
                        R1cp = small.tile([128, 256], BF16, tag="R1cp")
                        nc.scalar.copy(R1cp[:], R1ps[:, rsl])
                        blk = 2 * p + q
                        bsl = slice(blk * 128, (blk + 1) * 128)
                        nc.tensor.matmul(R2ps[:, bsl], R1cp[:, 0:128], Csc[:], start=True, stop=False)
                        nc.tensor.matmul(R2ps[:, bsl], R1cp[:, 128:256], Sscn[:], start=False, stop=True)
                        nc.vector.tensor_add(out_sb[:, bsl], R2ps[:, bsl], x_sb[:].bitcast(F32))

                nc.sync.dma_start(
                    out_t[i].rearrange("pq m f -> m pq f"),
                    out_sb[:].rearrange("p (pq f) -> p pq f", pq=4),
                )

    nc.compile()
    return nc


@functools.lru_cache(maxsize=2)
def _built(n_planes=NPL):
    return build_nc(n_planes)


def make_in_maps(x, k, alpha, n_planes=NPL, n_cores=N_CORES):
    consts = _host_consts()
    alpha_c = alpha.reshape(-1).astype(np.float64)  # [64]
    be = (1.0 / (1.0 + np.exp(-(alpha_c - 9.0))) + 1e-3).astype(np.float32)
    cpc = n_planes // 4  # channels per core
    in_maps = []
    for ci in range(n_cores):
        chs = slice(ci * cpc, (ci + 1) * cpc)
        xs = np.ascontiguousarray(x[:, chs].transpose(1, 0, 2, 3).reshape(n_planes, 128, 128))
        kt = np.ascontiguousarray(k[:, chs].transpose(1, 0, 3, 2).reshape(n_planes, KS, KS))
        be_pl = np.repeat(be[chs], 4)  # plane order: (c_loc, b)
        besb = np.broadcast_to(be_pl, (128, n_planes)).astype(np.float32).copy()
        m = {"xs": xs, "kt": kt, "besb": besb}
        m.update(consts)
        in_maps.append(m)
    return in_maps


def _assemble(res, n_planes=NPL, n_cores=N_CORES):
    out = np.empty((4, 64, 256, 256), np.float32)
    cpc = n_planes // 4
    for ci in range(n_cores):
        o = res.results[ci]["out"].reshape(cpc, 4, 2, 2, 128, 128)
        # [c_loc, b, p, q, m, n] -> [b, c_loc, m, p, n, q] -> [b, c_loc, 256, 256]
        o = o.transpose(1, 0, 4, 2, 5, 3).reshape(4, cpc, 256, 256)
        out[:, ci * cpc:(ci + 1) * cpc] = o
    return out


def kernel(x, k, alpha, sf=2, **_ignored):
    x = np.asarray(x, dtype=np.float32)
    k = np.asarray(k, dtype=np.float32)
    alpha = np.asarray(alpha, dtype=np.float32)
    assert int(sf) == 2 and x.shape == (4, 64, 128, 128) and k.shape == (4, 64, KS, KS)

    nc = _built(NPL)
    in_maps = make_in_maps(x, k, alpha)
    res = run_bass_kernel_spmd(nc, in_maps, core_ids=list(range(N_CORES)))
    return _assemble(res)


if __name__ == "__main__":
    rng = np.random.default_rng(0)
    x = rng.standard_normal((4, 64, 128, 128), dtype=np.float32)
    k = rng.random((4, 64, KS, KS), dtype=np.float32)
    alpha = np.zeros((1, 64, 1, 1), np.float32)
    out = kernel(x, k, alpha, 2)
    print("out", out.shape, out.dtype, float(np.abs(out).max()))


# revision 18
# speedup vs baseline: 2.1979x; 1.0658x over previous
"""Trainium2 Bass kernel for ConvReverseDataNet (USRNet-style FFT data step).

128-grid reformulation (per (b,c) plane, sf=2), validated in fp64/bf16 numpy:
  g   = DFT2_128(x)
  s_pq[a,b] = rolled_psf256[2a+p, 2b+q]   (parity subkernels of the 25x25 psf)
  Khat_pq = DFT2_128(s_pq) * e^{-2pi i(u p + v q)/128}  (twiddles folded into
            the small-DFT constants via positions (j+par)/2)
  W'  = sum_pq |Khat_pq|^2 ;  Y0' = sum_pq Khat_pq      (both on 128 grid)
  wt  = (1 - Y0') / (W' + be) ;  v = g * wt
  out[2m+p, 2n+q] = real(IDFT2_128(conj(Khat_pq) * v))[m,n] + x[m,n]

mm(stat, mov) = stat.T @ mov (contract over partition dim). Complex arrays
X stored as adjacent [Xr | Xs] blocks. Big-DFT matmuls run in float32r
(1 cycle/row at N>=256); the M/ifft stages run in bf16 (validated 3.2e-3).

Sharding: 256 (b,c) planes over 8 cores; core ci gets channels ci*8..ci*8+7.
Output is parity-planar [plane, 2p+q, m, n]; host interleaves.
"""

import functools
import sys

import numpy as np

if "/opt/trn_rl_repo" not in sys.path:
    sys.path.insert(0, "/opt/trn_rl_repo")

from concourse import bacc, bass, mybir, tile  # noqa: E402
from concourse.bass_utils import run_bass_kernel_spmd  # noqa: E402

F32 = mybir.dt.float32
F32R = mybir.dt.float32r
BF16 = mybir.dt.bfloat16
MULT = mybir.AluOpType.mult
ADD = mybir.AluOpType.add
ACT_COPY = mybir.ActivationFunctionType.Copy

N_CORES = 8
NPL = 32  # planes per core
KS = 25


def _host_consts():
    t = np.arange(KS)
    j = (t - 12) % 256            # position in rolled 256 grid
    par = t % 2                   # parity (row p / col q)
    bp = (j + par) // 2           # twiddle-folded 128-grid position

    u = np.arange(128)
    ang = 2 * np.pi * np.outer(bp, u) / 128  # [25,128]
    COSg = np.cos(ang).astype(np.float32)
    SINg = np.sin(ang).astype(np.float32)
    sel = [(par == 0).astype(np.float32)[:, None], (par == 1).astype(np.float32)[:, None]]

    c = {}
    c["WMOV"] = np.concatenate(
        [COSg * sel[0], COSg * sel[1], -SINg * sel[0], -SINg * sel[1]], axis=1
    )  # [25,512] -> A blocks [Ar0|Ar1|As0|As1]
    for p in (0, 1):
        c[f"ZC{p}"] = COSg * sel[p]
        c[f"ZS{p}"] = -SINg * sel[p]
        c[f"ZSn{p}"] = SINg * sel[p]

    th = 2 * np.pi * np.outer(u, u) / 128
    C1 = np.cos(th).astype(np.float32)
    S1 = np.sin(th).astype(np.float32)
    c["CS"] = np.concatenate([C1, S1], 1)
    c["CSn"] = np.concatenate([C1, -S1], 1)
    c["SnCn"] = np.concatenate([-S1, -C1], 1)
    c["ICSf"] = np.concatenate([C1, S1], 1)
    c["ISnCf"] = np.concatenate([-S1, C1], 1)
    c["Cscf"] = C1 / 16384.0
    c["Sscnf"] = -S1 / 16384.0
    return {n: np.ascontiguousarray(a, dtype=np.float32) for n, a in c.items()}


CONST_SHAPES = {
    "WMOV": [KS, 512],
    "ZC0": [KS, 128], "ZC1": [KS, 128],
    "ZS0": [KS, 128], "ZS1": [KS, 128],
    "ZSn0": [KS, 128], "ZSn1": [KS, 128],
    "CS": [128, 256], "CSn": [128, 256], "SnCn": [128, 256],
    "ICSf": [128, 256], "ISnCf": [128, 256],
    "Cscf": [128, 128], "Sscnf": [128, 128],
}


def build_nc(n_planes=NPL):
    nc = bacc.Bacc("TRN2", target_bir_lowering=False, debug=False, num_devices=N_CORES)

    xs_t = nc.dram_tensor("xs", [n_planes, 128, 128], F32R, kind="ExternalInput")
    kt_t = nc.dram_tensor("kt", [n_planes, KS, KS], F32R, kind="ExternalInput")
    be_t = nc.dram_tensor("besb", [128, n_planes], F32, kind="ExternalInput")
    const_t = {n: nc.dram_tensor(n, s, (F32 if n.endswith("f") else F32R), kind="ExternalInput") for n, s in CONST_SHAPES.items()}
    out_t = nc.dram_tensor("out", [n_planes, 4, 128, 128], F32, kind="ExternalOutput")

    X = mybir.AxisListType.X

    with tile.TileContext(nc) as tc:
        with (
            tc.tile_pool(name="cpool", bufs=1) as cpool,
            tc.tile_pool(name="small", bufs=4) as small,
            tc.tile_pool(name="med", bufs=3) as med,
            tc.tile_pool(name="psum", bufs=1, space="PSUM") as pp,
        ):
            cs = {}
            for n, s in CONST_SHAPES.items():
                cs[n] = cpool.tile(s, (F32 if n.endswith("f") else F32R), tag=n, name=f"c_{n}")
                nc.sync.dma_start(cs[n][:], const_t[n][:])
            besb = cpool.tile([128, n_planes], F32, tag="besb")
            nc.sync.dma_start(besb[:], be_t[:])

            # one-time bf16 conversions for the ifft constants
            ICS = cpool.tile([128, 256], BF16, tag="ICS")
            ISnC = cpool.tile([128, 256], BF16, tag="ISnC")
            Csc = cpool.tile([128, 128], BF16, tag="Csc")
            Sscn = cpool.tile([128, 128], BF16, tag="Sscn")
            nc.scalar.copy(ICS[:], cs["ICSf"][:])
            nc.scalar.copy(ISnC[:], cs["ISnCf"][:])
            nc.scalar.copy(Csc[:], cs["Cscf"][:])
            nc.scalar.copy(Sscn[:], cs["Sscnf"][:])

            for i in range(n_planes):
                # ---- A = kt^T-transform of psf columns ----
                kt_sb = small.tile([KS, KS], F32R, tag="kt_sb")
                nc.sync.dma_start(kt_sb[:], kt_t[i])
                Aps = pp.tile([KS, 512], F32, tag="Aps")
                nc.tensor.matmul(Aps[:], kt_sb[:], cs["WMOV"][:], start=True, stop=True)
                AX = med.tile([KS, 1024], F32R, tag="AX")
                nc.scalar.copy(AX[:, 0:512], Aps[:])
                nc.scalar.mul(AX[:, 512:1024], Aps[:], -1.0)
                AXv = AX[:].rearrange("c (b f) -> c b f", b=8)

                # ---- Khat per row-parity p: psum [Kr_p0|Kr_p1|Ks_p0|Ks_p1] ----
                Kps = []
                for p in (0, 1):
                    kp = pp.tile([128, 512], F32, tag="big512", bufs=2)
                    ZC, ZS = cs[f"ZC{p}"], cs[f"ZS{p}"]
                    nc.tensor.matmul(kp[:, 0:256], ZC[:], AX[:, 0:256], start=True, stop=False)
                    nc.tensor.matmul(kp[:, 0:256], ZS[:], AX[:, 768:1024], start=False, stop=True)
                    nc.tensor.matmul(kp[:, 256:512], ZC[:], AX[:, 256:512], start=True, stop=False)
                    nc.tensor.matmul(kp[:, 256:512], ZS[:], AX[:, 0:256], start=False, stop=True)
                    Kps.append(kp)

                # ---- T (bf16 Khat, both parities) ----
                Tall = med.tile([128, 1024], BF16, tag="Tall")
                nc.scalar.copy(Tall[:, 0:512], Kps[0][:])
                nc.scalar.copy(Tall[:, 512:1024], Kps[1][:])
                v8 = Tall[:].rearrange("p (P h q f) -> p P h q f", P=2, h=2, q=2)

                # ---- Y0' = sum_pq Khat_pq via gpsimd folds ----
                y0a = small.tile([128, 256], F32, tag="y0a")
                y0b = small.tile([128, 256], F32, tag="y0b")
                Y0f = small.tile([128, 256], F32, tag="Y0f")  # [Y0r | Y0s]
                hf = lambda ap: ap.rearrange("p (h f) -> p h f", h=2)  # noqa: E731
                nc.gpsimd.tensor_add(hf(y0a[:]), v8[:, 0, :, 0, :], v8[:, 0, :, 1, :])
                nc.gpsimd.tensor_add(hf(y0b[:]), v8[:, 1, :, 0, :], v8[:, 1, :, 1, :])
                nc.gpsimd.tensor_add(Y0f[:], y0a[:], y0b[:])
                sq0 = med.tile([128, 512], F32, tag="sq0")
                sq1 = med.tile([128, 512], F32, tag="sq1")
                nc.scalar.square(sq0[:], Kps[0][:])
                nc.scalar.square(sq1[:], Kps[1][:])
                sqs = med.tile([128, 512], F32, tag="sqs")
                nc.gpsimd.tensor_add(sqs[:], sq0[:], sq1[:])
                sqh = small.tile([128, 256], F32, tag="sqh")
                nc.vector.tensor_add(sqh[:], sqs[:, 0:256], sqs[:, 256:512])
                Wt = small.tile([128, 128], F32, tag="Wt")
                nc.vector.tensor_add(Wt[:], sqh[:, 0:128], sqh[:, 128:256])

                # ---- wt = (1 - Y0')/(W' + be) ----
                den = small.tile([128, 128], F32, tag="den")
                nc.vector.tensor_scalar_add(den[:], Wt[:], besb[:, i:i + 1])
                dinv = small.tile([128, 128], F32, tag="dinv")
                nc.vector.reciprocal_approx_fast(dinv[:], den[:])
                num = small.tile([128, 128], F32, tag="num")
                nc.scalar.activation(num[:], Y0f[:, 0:128], ACT_COPY, bias=1.0, scale=-1.0)
                wtile = small.tile([128, 256], BF16, tag="wtile")  # [wr | ws]
                nc.vector.tensor_mul(wtile[:, 0:128], num[:], dinv[:])
                nc.vector.scalar_tensor_tensor(wtile[:, 128:256], Y0f[:, 128:256], -1.0, dinv[:], MULT, MULT)

                # ---- g = DFT2_128(x) ----
                x_sb = small.tile([128, 128], F32R, tag="x_sb")
                nc.sync.dma_start(x_sb[:], xs_t[i])
                Pps = pp.tile([128, 256], F32, tag="Pps")
                nc.tensor.matmul(Pps[:], x_sb[:], cs["CS"][:], start=True, stop=True)
                Pcp = small.tile([128, 256], F32R, tag="Pcp")
                nc.scalar.copy(Pcp[:], Pps[:])
                gps = pp.tile([128, 256], F32, tag="gps")
                nc.tensor.matmul(gps[:], Pcp[:, 0:128], cs["CSn"][:], start=True, stop=False)
                nc.tensor.matmul(gps[:], Pcp[:, 128:256], cs["SnCn"][:], start=False, stop=True)
                gb = small.tile([128, 256], BF16, tag="gb")
                nc.scalar.copy(gb[:], gps[:])

                # ---- v = g * wt (bf16): t_a=[gr wr|gr ws], t_b=[gs wr|gs ws] ----
                b2 = lambda ap: ap.rearrange("p (b f) -> p b f", b=2)  # noqa: E731
                t_a = small.tile([128, 256], BF16, tag="t_a")
                t_b = small.tile([128, 256], BF16, tag="t_b")
                gr2 = gb[:, 0:128].unsqueeze(1).broadcast_to([128, 2, 128])
                gs2 = gb[:, 128:256].unsqueeze(1).broadcast_to([128, 2, 128])
                nc.vector.tensor_mul(b2(t_a[:]), gr2, b2(wtile[:]))
                nc.vector.tensor_mul(b2(t_b[:]), gs2, b2(wtile[:]))
                vr = small.tile([128, 128], BF16, tag="vr")
                vs = small.tile([128, 128], BF16, tag="vs")
                nc.vector.tensor_sub(vr[:], t_a[:, 0:128], t_b[:, 128:256])
                nc.vector.tensor_add(vs[:], t_a[:, 128:256], t_b[:, 0:128])
                vr4 = vr[:].unsqueeze(1).unsqueeze(1).broadcast_to([128, 2, 2, 128])
                vs4 = vs[:].unsqueeze(1).unsqueeze(1).broadcast_to([128, 2, 2, 128])

                # ---- M_pq = conj(Khat_pq) * v (wide ops) ----
                KRv = v8[:, :, 0, :, :]   # [128, P, q, 128] real parts
                KSv = v8[:, :, 1, :, :]   # imag parts
                b4 = lambda ap: ap.rearrange("p (P q f) -> p P q f", P=2, q=2)  # noqa: E731
                m1 = med.tile([128, 512], BF16, tag="m1")
                m2 = med.tile([128, 512], BF16, tag="m2")
                MR = med.tile([128, 512], BF16, tag="MR")
                nc.vector.tensor_mul(b4(m1[:]), KRv, vr4)
                nc.vector.tensor_mul(b4(m2[:]), KSv, vs4)
                nc.vector.tensor_add(MR[:], m1[:], m2[:])
                m3 = med.tile([128, 512], BF16, tag="m3")
                m4 = med.tile([128, 512], BF16, tag="m4")
                MS = med.tile([128, 512], BF16, tag="MS")
                nc.vector.tensor_mul(b4(m3[:]), KRv, vs4)
                nc.vector.tensor_mul(b4(m4[:]), KSv, vr4)
                nc.vector.tensor_sub(MS[:], m3[:], m4[:])

                # ---- ifft + x ; store ----
                out_sb = med.tile([128, 512], F32, tag="out_sb")
                R2ps = pp.tile([128, 512], F32, tag="R2ps")
                for p in (0, 1):
                    R1ps = pp.tile([128, 512], F32, tag="R1ps", bufs=2)
                    for q in (0, 1):
                        blk = 2 * p + q
                        rsl = slice(q * 256, (q + 1) * 256)
                        bsl = slice(blk * 128, (blk + 1) * 128)
                        nc.tensor.matmul(R1ps[:, rsl], MR[:, bsl], ICS[:], start=True, stop=False)
                        nc.tensor.matmul(R1ps[:, rsl], MS[:, bsl], ISnC[:], start=False, stop=True)
                        R1cp = small.tile([128, 256], BF16, tag="R1cp")
                        nc.scalar.copy(R1cp[:], R1ps[:, rsl])
                        nc.tensor.matmul(R2ps[:, bsl], R1cp[:, 0:128], Csc[:], start=True, stop=False)
                        nc.tensor.matmul(R2ps[:, bsl], R1cp[:, 128:256], Sscn[:], start=False, stop=True)
                        nc.vector.tensor_add(out_sb[:, bsl], R2ps[:, bsl], x_sb[:].bitcast(F32))

                nc.sync.dma_start(
                    out_t[i].rearrange("pq m f -> m pq f"),
                    out_sb[:].rearrange("p (pq f) -> p pq f", pq=4),
                )

    nc.compile()
    return nc


@functools.lru_cache(maxsize=2)
def _built(n_planes=NPL):
    return build_nc(n_planes)


def make_in_maps(x, k, alpha, n_planes=NPL, n_cores=N_CORES):
    consts = _host_consts()
    alpha_c = alpha.reshape(-1).astype(np.float64)  # [64]
    be = (1.0 / (1.0 + np.exp(-(alpha_c - 9.0))) + 1e-3).astype(np.float32)
    cpc = n_planes // 4  # channels per core
    in_maps = []
    for ci in range(n_cores):
        chs = slice(ci * cpc, (ci + 1) * cpc)
        xs = np.ascontiguousarray(x[:, chs].transpose(1, 0, 2, 3).reshape(n_planes, 128, 128))
        kt = np.ascontiguousarray(k[:, chs].transpose(1, 0, 3, 2).reshape(n_planes, KS, KS))
        be_pl = np.repeat(be[chs], 4)  # plane order: (c_loc, b)
        besb = np.broadcast_to(be_pl, (128, n_planes)).astype(np.float32).copy()
        m = {"xs": xs, "kt": kt, "besb": besb}
        m.update(consts)
        in_maps.append(m)
    return in_maps


def _assemble(res, n_planes=NPL, n_cores=N_CORES):
    out = np.empty((4, 64, 256, 256), np.float32)
    cpc = n_planes // 4
    for ci in range(n_cores):
        o = res.results[ci]["out"].reshape(cpc, 4, 2, 2, 128, 128)
        # [c_loc, b, p, q, m, n] -> [b, c_loc, m, p, n, q] -> [b, c_loc, 256, 256]
        o = o.transpose(1, 0, 4, 2, 5, 3).reshape(4, cpc, 256, 256)
        out[:, ci * cpc:(ci + 1) * cpc] = o
    return out


def kernel(x, k, alpha, sf=2, **_ignored):
    x = np.asarray(x, dtype=np.float32)
    k = np.asarray(k, dtype=np.float32)
    alpha = np.asarray(alpha, dtype=np.float32)
    assert int(sf) == 2 and x.shape == (4, 64, 128, 128) and k.shape == (4, 64, KS, KS)

    nc = _built(NPL)
    in_maps = make_in_maps(x, k, alpha)
    res = run_bass_kernel_spmd(nc, in_maps, core_ids=list(range(N_CORES)))
    return _assemble(res)


if __name__ == "__main__":
    rng = np.random.default_rng(0)
    x = rng.standard_normal((4, 64, 128, 128), dtype=np.float32)
    k = rng.random((4, 64, KS, KS), dtype=np.float32)
    alpha = np.zeros((1, 64, 1, 1), np.float32)
    out = kernel(x, k, alpha, 2)
    print("out", out.shape, out.dtype, float(np.abs(out).max()))
